# revision 1
# baseline (speedup 1.0000x reference)
"""Trainium2 Bass kernel for nn_BaseRecommender (masked top-k recommendation).

Strategy (hardcoded, self-contained):
  - Shard the item embedding table column-wise (item dim) across 8 cores:
    12500 items/core, zero-padded to 13312 = 13 matmul chunks x 1024.
    Replicate u_e = all_embed[user_list] (gathered + transposed on host).
  - Per core, per 128-row tile: f32r matmuls -> PSUM [128,1024] chunks.
    The score matrix is REDUCED on the fly instead of scanned: chunks are
    folded pairwise (elementwise max) into a bf16 [128, 7168] buffer.  For
    3 of the 6 pairs the scalar engine copies chunk A to SBUF (bf16) and
    the DVE folds that copy against chunk B still in PSUM (drain + fold in
    one op; the ISA forbids two PSUM operands); for the rest the scalar
    engine copies both chunks and the DVE folds in bf16 (2x mode).  This
    split balances the scalar drain against the DVE fold mass given the
    cayman DVE drain erratum (each DVE op is followed by a pipeline-flush
    of ~op_duration-266ns).  Further DVE halving folds shrink 7168 -> 448
    slots; one max8 + max_index extracts the top-8 slots per row.  A slot
    j covers item columns {2048*(p//1024) + p%1024 + {0,1024} : p = j +
    448*m, m<16} (32 columns).
  - Host: exact rescore of all 8*8*32 member candidates per row, exact
    masked scores for global item cols [0, 1024) (the only range the
    reference ever masks, since it keeps only item_idx < BATCH), merge,
    tie-aware top-k (stable, lower index first, matching jax.lax.top_k).
    A guard recomputes a core's full shard for any row where the core's
    8th slot value + noise margin could still reach the row's 20th value
    (covers slot-collision truncation, f32r matmul noise and bf16
    rounding), so the result is exact.
"""

import os
import sys

import numpy as np

try:
    import concourse  # noqa: F401
except ImportError:
    for _p in ("/opt/trn_rl_repo", os.path.expanduser("~/.axon_site/_ro/trn_rl_repo")):
        if os.path.isdir(_p):
            sys.path.insert(0, _p)
            try:
                import concourse  # noqa: F401

                break
            except ImportError:
                sys.path.remove(_p)

N_USERS = 100000
N_ITEMS = 100000
EMB = 64
BATCH = 1024
K = 20
NEG = -100000.0
NCORES = 8
ISHARD = N_ITEMS // NCORES  # 12500 items per core
PCH = 1024  # matmul/psum chunk (columns)
NCH = 13  # chunks per core (12 pair-folded + 1 copied straight into sfold)
IPAD = NCH * PCH  # 13312
FOLD0 = 7 * PCH  # sfold width (6 pair slots + 1 direct chunk)
NSLOT = 448  # final fold width (slots per row tile)
MEMB = 2 * FOLD0 // NSLOT  # 32 (logical) item columns per slot
N_MIX = 3  # pairs drained as: scalar copies A, DVE folds the copy against
# the PSUM-resident B (rest: two scalar copies + DVE bf16 fold)
ROWT = 128
NROWT = BATCH // ROWT  # 8 row tiles
HOST_COLS = 1024  # item columns [0, HOST_COLS) are scored on host (mask range)
MARGIN_EPS = 4e-3  # guard margin: f32r matmul noise + bf16 rounding

_compiled = None


def _build_bass(loop_n=1):
    """Build the per-core Bass program. loop_n > 1 repeats the compute loop
    (hardware For_i) for differential HW timing; loads happen once."""
    from concourse import bacc
    import concourse.mybir as mybir
    from concourse.tile import TileContext

    F32 = mybir.dt.float32
    F32R = mybir.dt.float32r
    BF16 = mybir.dt.bfloat16
    U16 = mybir.dt.uint16
    MAX = mybir.AluOpType.max

    nc = bacc.Bacc("TRN2", target_bir_lowering=False, debug=False, num_devices=NCORES)
    u_t = nc.dram_tensor("u_t", [EMB, BATCH], F32R, kind="ExternalInput")
    i_t = nc.dram_tensor("i_t", [EMB, IPAD], F32R, kind="ExternalInput")
    cv = nc.dram_tensor("cv", [BATCH, 8], BF16, kind="ExternalOutput")
    ci = nc.dram_tensor("ci", [BATCH, 8], U16, kind="ExternalOutput")

    with TileContext(nc) as tc:
        with (
            tc.tile_pool(name="consts", bufs=1) as consts,
            tc.tile_pool(name="psum", bufs=2, space="PSUM") as psum,
            tc.tile_pool(name="raw", bufs=2) as rawp,
            tc.tile_pool(name="fold", bufs=2) as foldp,
            tc.tile_pool(name="cand", bufs=2) as cand,
        ):
            u_sb = consts.tile([EMB, BATCH], F32R, tag="u_sb")
            nc.sync.dma_start(u_sb[:], u_t[:])
            i_sb = []
            for c in range(NCH):
                t = consts.tile([EMB, PCH], F32R, tag=f"i_sb{c}")
                nc.sync.dma_start(t[:], i_t[:, c * PCH : (c + 1) * PCH])
                i_sb.append(t)

            npairs = (NCH - 1) // 2  # 6
            nsc = N_MIX + 2 * (npairs - N_MIX)  # sraw slots (1024 each)

            def body():
                for rt in range(NROWT):
                    lhs = u_sb[:, rt * ROWT : (rt + 1) * ROWT]
                    sfold = foldp.tile([ROWT, FOLD0], BF16, tag="sfold")
                    sraw = rawp.tile([ROWT, nsc * PCH], BF16, tag="sraw")
                    fbuf = foldp.tile([ROWT, FOLD0 // 2], BF16, tag="fbuf")
                    v8 = cand.tile([ROWT, 8], BF16, tag="v8")
                    i8 = cand.tile([ROWT, 8], U16, tag="i8")

                    def pstile(c):
                        t = psum.tile([ROWT, PCH], F32, tag=f"ps{c % 2}")
                        for h in (0, 512):
                            nc.tensor.matmul(
                                t[:, h : h + 512],
                                lhs,
                                i_sb[c][:, h : h + 512],
                                start=True,
                                stop=True,
                            )
                        return t

                    sr = 0  # next sraw slot
                    for q in range(npairs):  # chunk pair (2q, 2q+1)
                        dst = sfold[:, q * PCH : (q + 1) * PCH]
                        psA, psB = pstile(2 * q), pstile(2 * q + 1)
                        if q < N_MIX:
                            ra = sraw[:, sr * PCH : (sr + 1) * PCH]
                            sr += 1
                            nc.scalar.copy(ra, psA[:])
                            # DVE drains chunk B straight from PSUM via the fold
                            nc.vector.tensor_tensor(dst, ra, psB[:], op=MAX)
                        else:
                            ra = sraw[:, sr * PCH : (sr + 1) * PCH]
                            rb = sraw[:, (sr + 1) * PCH : (sr + 2) * PCH]
                            sr += 2
                            nc.scalar.copy(ra, psA[:])
                            nc.scalar.copy(rb, psB[:])
                            nc.vector.tensor_tensor(dst, ra, rb, op=MAX)
                    # leftover chunk (NCH-1): straight bf16 copy into sfold
                    psL = pstile(NCH - 1)
                    nc.scalar.copy(sfold[:, FOLD0 - PCH : FOLD0], psL[:])

                    # DVE halving folds FOLD0 -> NSLOT (into fbuf, then in place)
                    n = FOLD0
                    cur = sfold
                    while n > NSLOT:
                        h = n // 2
                        dst = fbuf if cur is sfold else cur
                        nc.vector.tensor_tensor(
                            dst[:, 0:h], cur[:, 0:h], cur[:, h:n], op=MAX
                        )
                        cur = dst
                        n = h
                    nc.vector.max(v8[:], cur[:, 0:NSLOT])
                    nc.vector.max_index(i8[:], v8[:], cur[:, 0:NSLOT])
                    nc.sync.dma_start(cv[rt * ROWT : (rt + 1) * ROWT, :], v8[:])
                    nc.sync.dma_start(ci[rt * ROWT : (rt + 1) * ROWT, :], i8[:])

            if loop_n == 1:
                body()
            else:
                with tc.For_i(0, loop_n, 1):
                    body()

    nc.compile()
    return nc


def _get_compiled():
    global _compiled
    if _compiled is None:
        _compiled = _build_bass()
    return _compiled


def run_device(u_t, i_t_shards, trace=False, **kwargs):
    from concourse.bass_utils import run_bass_kernel_spmd

    nc = _get_compiled()
    in_maps = [{"u_t": u_t, "i_t": i_t_shards[s]} for s in range(NCORES)]
    return run_bass_kernel_spmd(nc, in_maps, list(range(NCORES)), trace=trace, **kwargs)


def make_device_inputs(all_embed, user_list):
    all_embed = np.asarray(all_embed, dtype=np.float32)
    user_list = np.asarray(user_list)
    u_e = all_embed[user_list.astype(np.int64)]  # [BATCH, EMB]
    i_e = all_embed[N_USERS:]  # [I, EMB]
    u_t = np.ascontiguousarray(u_e.T)  # [EMB, BATCH]
    i_t_shards = []
    for s in range(NCORES):
        sh = np.zeros((EMB, IPAD), dtype=np.float32)
        sh[:, :ISHARD] = i_e[s * ISHARD : (s + 1) * ISHARD].T
        i_t_shards.append(sh)
    return u_e, i_e, u_t, i_t_shards


def _member_table():
    """[NSLOT, MEMB] local item indices covered by each final slot."""
    j = np.arange(NSLOT)[:, None]  # slot
    m = np.arange(MEMB // 2)[None, :]  # fold position within slot
    p = j + NSLOT * m  # sfold position [NSLOT, MEMB/2]
    q, t = p // PCH, p % PCH
    lo = 2048 * q + t
    return np.concatenate([lo, lo + PCH], axis=1)  # [NSLOT, MEMB]


MEMBER_TABLE = _member_table()


def _mask_host_scores(s0, pos_pad):
    """Reference masking semantics on the host-scored region: only valid
    positives with local item index < BATCH (== HOST_COLS) are masked."""
    pos_pad = np.asarray(pos_pad)
    item_idx = pos_pad.astype(np.int64) - N_USERS
    valid = (pos_pad >= 0) & (item_idx < HOST_COLS)
    r, c = np.nonzero(valid)
    np.minimum.at(s0, (r, item_idx[r, c]), np.float32(NEG))
    return s0


def postprocess(results, u_e, i_e, pos_pad):
    """Expand per-core top-8 slots to member candidates, rescore exactly,
    merge with the host-masked region, select the exact global top-K."""
    slot_v = np.stack(
        [results[s]["cv"].astype(np.float32) for s in range(NCORES)]
    )  # [S, B, 8]
    slot_i = np.stack(
        [results[s]["ci"].astype(np.int64) for s in range(NCORES)]
    )  # [S, B, 8]

    mem_local = MEMBER_TABLE[slot_i]  # [S, B, 8, MEMB]
    glob = np.arange(NCORES, dtype=np.int64)[:, None, None, None] * ISHARD + mem_local
    ok = (mem_local < ISHARD) & (glob >= HOST_COLS)

    cand_g = glob.transpose(1, 0, 2, 3).reshape(BATCH, -1)  # [B, 8*8*MEMB]
    cand_ok = ok.transpose(1, 0, 2, 3).reshape(BATCH, -1)
    safe_g = np.where(cand_ok, cand_g, 0)
    NC = cand_g.shape[1]
    cand_v = np.empty((BATCH, NC), dtype=np.float32)
    step = 512  # chunk the gather-einsum to bound memory
    for c0 in range(0, NC, step):
        cand_v[:, c0 : c0 + step] = np.einsum(
            "re,rce->rc", u_e, i_e[safe_g[:, c0 : c0 + step]], optimize=True
        ).astype(np.float32)
    cand_v[~cand_ok] = -np.inf
    cand_g = np.where(cand_ok, cand_g, -1)

    # Host-exact scores for the maskable region (global item cols [0, 1024)).
    s0 = u_e @ i_e[:HOST_COLS].T  # [BATCH, HOST_COLS] float32
    s0 = _mask_host_scores(s0, pos_pad)

    all_v = np.concatenate([s0, cand_v], axis=1)
    all_g = np.concatenate(
        [
            np.broadcast_to(np.arange(HOST_COLS, dtype=np.int64), (BATCH, HOST_COLS)),
            cand_g,
        ],
        axis=1,
    )

    # Arrange columns in ascending global index so a stable sort on -value
    # reproduces the reference's tie order (lower index first).
    ordg = np.argsort(
        np.where(all_g < 0, np.int64(1) << 40, all_g), axis=1, kind="stable"
    )
    rows = np.arange(BATCH)[:, None]
    all_v = all_v[rows, ordg]
    all_g = all_g[rows, ordg]

    order = np.argsort(-all_v, axis=1, kind="stable")[:, :K]
    out_val = all_v[rows, order]
    out_idx = all_g[rows, order]
    v20 = out_val[:, K - 1]

    # Guard: a core's 8th slot value (+noise margin) reaching the row's 20th
    # means that core may hide better candidates -> recompute its full shard.
    scale = np.maximum(np.abs(out_val[:, 0]), 1.0)
    margin = MARGIN_EPS * scale  # [B]
    trig = slot_v[:, :, 7] + margin[None, :] >= v20[None, :]  # [S, B]
    for r in np.nonzero(trig.any(axis=0))[0].tolist():
        cores = np.nonzero(trig[:, r])[0].tolist()
        keep = np.ones(all_g.shape[1], dtype=bool)
        vals_ext, gidx_ext = [], []
        for s in cores:
            lo = max(s * ISHARD, HOST_COLS)
            hi = (s + 1) * ISHARD
            keep &= ~((all_g[r] >= lo) & (all_g[r] < hi))
            vals_ext.append((i_e[lo:hi] @ u_e[r]).astype(np.float32))
            gidx_ext.append(np.arange(lo, hi, dtype=np.int64))
        keep &= all_g[r] >= 0
        vals = np.concatenate([all_v[r][keep]] + vals_ext)
        gidx = np.concatenate([all_g[r][keep]] + gidx_ext)
        o = np.lexsort((gidx, -vals.astype(np.float64)))[:K]
        out_val[r] = vals[o].astype(np.float32)
        out_idx[r] = gidx[o]
    return out_idx.astype(np.int32) + N_USERS, out_val


def kernel(all_embed, pos_pad, user_list, k):
    pos_pad = np.asarray(pos_pad)
    k = int(k)
    assert k == K, f"kernel hardcoded for k={K}, got {k}"
    u_e, i_e, u_t, i_t_shards = make_device_inputs(all_embed, user_list)
    res = run_device(u_t, i_t_shards)
    return postprocess(res.results, u_e, i_e, pos_pad)

